# revision 28
# baseline (speedup 1.0000x reference)
"""Deformable alignment fusion kernel for TRN2, 8-core data-parallel.

Math (per batch b):
  cat    = concat([low, high], ch)                       # (256, H, W)
  offset = conv3x3(cat, w_off) + b_off                   # (18, H, W)  (dy,dx)*9 taps
  aligned= deform_conv(low, offset, w_def) + b_def       # (128, H, W)
  gate   = sigmoid(w_mod @ cat + b_mod)                  # (128, H, W)
  out    = aligned * gate + high

Sharding: core i handles batch b = i//2, rows [64*(i%2), 64*(i%2)+64).

Device algorithm per core (channel-major, fp16 matmul operands):
 - offset conv: direct 3x3 conv as 18 accumulating matmuls per 512-px chunk.
 - bilinear sampling in "monomial" form: S = P0 + wx*P1 + wy*P2 + wx*wy*P3
   where P0..P3 are the value / x-diff / y-diff / xy-diff planes of the
   (guard-padded) low image, all gathered at the single flat index
   i0 = floor(py)*136 + floor(px) from a host-prepared pixel-major
   4-plane table via dma_gather(transpose=True) -> channel-major tiles.
   Gathers round-robin over 4 SWDGE queues so descriptor generation runs
   on all four Q7 core pairs concurrently.
 - per-pixel weights (wx, wy, wx*wy) are broadcast to 128 partitions with
   a 1-contraction ones-matmul into PSUM; a single DVE op multiplies the
   three difference planes in place against the PSUM-resident weights.
 - the deform conv contraction folds the monomial sum into PSUM:
   psum += w_def_k.T @ G0 ; += w_def_k.T @ (wx*G1) ; ...  (4 matmuls/tap)
 - gate: two 1x1 matmuls + Sigmoid on the scalar engine.
"""

import numpy as np

import concourse.bass as bass
import concourse.tile as tile
from concourse import bacc, mybir
from concourse.bass import ts

dt = mybir.dt
F16 = dt.float16
F32 = dt.float32
I16 = dt.int16
Alu = mybir.AluOpType
Act = mybir.ActivationFunctionType

B, C, H, W = 4, 128, 128, 128
GP = 4                 # guard pad for sampling
HP = H + 2 * GP        # 136
NP = HP * HP           # 18496 padded pixels
HR = 64                # rows per core
NPIX = HR * W          # 8192 pixels per core
NQ = 4                 # quarters per core
QP = NPIX // NQ        # 2048 pixels per quarter
QR = QP // W           # 16 rows per quarter
CLAMP_HI = float(H + 2 * GP - 2)  # 134.0 : floor+1 stays inside padded image
MAGIC_A = 8388608.0 - 0.5
MAGIC_B = 8388608.0

# offset channel regrouping: rows 0..8 = dy(tap), rows 9..17 = dx(tap)
PERM = [2 * k for k in range(9)] + [2 * k + 1 for k in range(9)]


def _ap(t, offset, dims):
    """Raw AP on the same tensor as AP `t`, with explicit [step, count] dims."""
    return bass.AP(tensor=t.tensor, offset=t.offset + offset, ap=list(dims))


def build_program():
    nc = bacc.Bacc("TRN2", debug=False, num_swdge_queues=4)

    io = {}

    def din(name, shape, d):
        io[name] = nc.dram_tensor(name, shape, d, kind="ExternalInput").ap()
        return io[name]

    din("src4", [NP, 512], F16)           # [pix, (4 planes x 128 ch)]
    din("lowp", [128, 66 * 130], F16)     # rows h0-1..h1+1, W-padded by 1
    din("highp", [128, 66 * 130], F16)
    din("highc", [128, NPIX], F32)        # center high rows, f32
    din("w_off_t", [2, 3, 3, 128, 18], F16)
    din("w_def_t", [9, 128, 128], F16)
    din("w_mod_t", [2, 128, 128], F16)
    din("b_off_g", [18, 1], F32)
    din("b_def_c", [128, 1], F32)
    din("b_mod_c", [128, 1], F32)
    din("base_w2", [18, NQ, QP], F16)     # sampling bases, wrapped cols, per quarter
    io["idx_scr"] = nc.dram_tensor("idx_scr", [36, QP], I16, kind="Internal").ap()
    out_d = nc.dram_tensor("out", [128, NPIX], F32, kind="ExternalOutput").ap()

    with tile.TileContext(nc) as tc:
        trace_kernel(tc, io, out_d)

    nc.compile()
    return nc


def trace_kernel(tc, io, out_d):
    nc = tc.nc
    from contextlib import ExitStack

    ctx = ExitStack()
    consts = ctx.enter_context(tc.tile_pool(name="consts", bufs=1))
    npool = ctx.enter_context(tc.tile_pool(name="narrow", bufs=1))
    spool = ctx.enter_context(tc.tile_pool(name="small", bufs=2))
    s1pool = ctx.enter_context(tc.tile_pool(name="small1", bufs=2))
    stgp = ctx.enter_context(tc.tile_pool(name="stg", bufs=2))

    # ---------------- constants to SBUF ----------------
    w_off_sb = consts.tile([128, 2, 3, 3, 18], F16)
    nc.sync.dma_start(
        w_off_sb[:], io["w_off_t"].rearrange("cb ky kx c o -> c cb ky kx o")
    )
    w_def_sb = consts.tile([128, 9, 128], F16)
    nc.sync.dma_start(w_def_sb[:], io["w_def_t"].rearrange("k c o -> c k o"))
    w_mod_sb = consts.tile([128, 2, 128], F16)
    nc.sync.dma_start(w_mod_sb[:], io["w_mod_t"].rearrange("cb c o -> c cb o"))
    b_off_sb = consts.tile([18, 1], F32)
    nc.sync.dma_start(b_off_sb[:], io["b_off_g"])
    b_def_sb = consts.tile([128, 1], F32)
    nc.sync.dma_start(b_def_sb[:], io["b_def_c"])
    b_mod_sb = consts.tile([128, 1], F32)
    nc.sync.dma_start(b_mod_sb[:], io["b_mod_c"])
    ones_sb = consts.tile([1, 128], F16)
    nc.vector.memset(ones_sb[:], 1.0)
    gate_sb = npool.tile([128, NPIX], F16, tag="gate")

    imgpool = ctx.enter_context(tc.tile_pool(name="imgs", bufs=1))
    lowp_sb = imgpool.tile([128, 66, 130], F16)
    nc.sync.dma_start(lowp_sb[:], io["lowp"].rearrange("c (h w) -> c h w", h=66))
    highp_sb = imgpool.tile([128, 66, 130], F16)
    nc.sync.dma_start(highp_sb[:], io["highp"].rearrange("c (h w) -> c h w", h=66))
    base_sb = imgpool.tile([18, NQ, QP], F16)
    nc.sync.dma_start(base_sb[:], io["base_w2"])

    # per-quarter narrow tiles (ping-pong: AB(q+1) overlaps DE(q))
    qpool = ctx.enter_context(tc.tile_pool(name="qnarrow", bufs=2))
    abpool = ctx.enter_context(tc.tile_pool(name="abscratch", bufs=1))
    qtiles = {}

    # ---------------- stage C: gate (head) ----------------
    gatectx = ExitStack()
    gpsum = gatectx.enter_context(tc.tile_pool(name="ps_gate", bufs=2, space="PSUM"))
    for ch in range(16):
        r0 = ch * 4
        psg = gpsum.tile([128, 512], F32, tag="gateps")
        for cb in range(2):
            pad = lowp_sb if cb == 0 else highp_sb
            nc.tensor.matmul(
                psg[:],
                lhsT=w_mod_sb[:, cb, :],
                rhs=pad[:, 1 + r0 : 1 + r0 + 4, 1:129],
                start=(cb == 0),
                stop=(cb == 1),
            )
        nc.scalar.activation(
            out=gate_sb[:, ts(ch, 512)], in_=psg[:],
            func=Act.Sigmoid, bias=b_mod_sb[:], scale=1.0,
        )
    gatectx.close()

    apool = ctx.enter_context(tc.tile_pool(name="ps_off", bufs=1, space="PSUM"))

    def stage_AB(q):
        """Offset conv + index/frac math for quarter q."""
        frac3 = qpool.tile([9, 3, QP], F16, tag="frac3")  # wx | wy | wx*wy, natural
        idxr = qpool.tile([128, 9, 128], I16, tag="idxr")
        qtiles[q] = (frac3, idxr)
        frac_nat = frac3[:].rearrange("r m (s p) -> r m p s", p=16)
        pos = abpool.tile([18, QP], F32, tag="pos")   # rows 0-8 py, 9-17 px
        posx = abpool.tile([9, QP], F32, tag="posx")
        ftmpy = abpool.tile([9, QP], F32, tag="ftmpy")
        ftmpx = abpool.tile([9, QP], F32, tag="ftmpx")
        idx16 = abpool.tile([9, QP], I16, tag="idx16")
        pos_wr = pos[:].rearrange("r (p s) -> r p s", p=16)
        posx_wr = posx[:].rearrange("r (p s) -> r p s", p=16)
        fty_wr = ftmpy[:].rearrange("r (p s) -> r p s", p=16)
        ftx_wr = ftmpx[:].rearrange("r (p s) -> r p s", p=16)
        for cc in range(4):
            ps = apool.tile([18, 512], F32, tag="offps")
            r0 = q * QR + cc * 4
            n_mm = 0
            for cb in range(2):
                pad = lowp_sb if cb == 0 else highp_sb
                for ky in range(3):
                    for kx in range(3):
                        nc.tensor.matmul(
                            ps[:],
                            lhsT=w_off_sb[:, cb, ky, kx, :],
                            rhs=pad[:, r0 + ky : r0 + ky + 4, kx : kx + 128],
                            start=(n_mm == 0),
                            stop=(n_mm == 17),
                        )
                        n_mm += 1
            # evacuate with wrapped reorder: pos = psum + b_off + base
            ps3 = ps[:].rearrange("r (s p) -> r p s", p=16)  # [18, 16, 32]
            nc.vector.scalar_tensor_tensor(
                out=pos_wr[:, :, cc * 32 : cc * 32 + 32],
                in0=ps3,
                scalar=b_off_sb[:],
                in1=base_sb[:].rearrange("r q (p s) -> r q p s", p=16)[
                    :, q, :, cc * 32 : cc * 32 + 32
                ],
                op0=Alu.add, op1=Alu.add,
            )
        # move px rows to a partition-0-based tile (compute ops cannot
        # start at partition 9)
        nc.sync.dma_start(posx[:], pos[9:18, :])
        # clamp in place
        nc.vector.tensor_scalar(pos[0:9, :], pos[0:9, :], 0.0, CLAMP_HI, Alu.max, Alu.min)
        nc.vector.tensor_scalar(posx[:], posx[:], 0.0, CLAMP_HI, Alu.max, Alu.min)
        # floor via round-to-nearest magic (exact-int result in f32)
        nc.vector.tensor_scalar(ftmpy[:], pos[0:9, :], MAGIC_A, MAGIC_B, Alu.add, Alu.subtract)
        nc.vector.tensor_scalar(ftmpx[:], posx[:], MAGIC_A, MAGIC_B, Alu.add, Alu.subtract)
        # fracs, written in natural column order via permuted APs
        nc.vector.tensor_tensor(frac_nat[:, 0], posx_wr[:], ftx_wr[:], Alu.subtract)
        nc.vector.tensor_tensor(frac_nat[:, 1], pos_wr[0:9], fty_wr[:], Alu.subtract)
        nc.vector.tensor_tensor(frac3[:, 2, :], frac3[:, 0, :], frac3[:, 1, :], Alu.mult)
        # flat index i0 = fy*136 + fx (exact in f32, cast rounds exactly)
        nc.vector.scalar_tensor_tensor(
            out=idx16[:], in0=ftmpy[:], scalar=float(HP), in1=ftmpx[:],
            op0=Alu.mult, op1=Alu.add,
        )
        # replicate wrapped idx rows to all 8 Q7 core groups via DRAM
        nc.sync.dma_start(_ap(io["idx_scr"], 9 * q * QP, [[QP, 9], [1, QP]]), idx16[:])
        for k in range(9):
            rep_ap = _ap(
                io["idx_scr"], (9 * q + k) * QP,
                [[0, 8], [128, 16], [1, 128]],
            )
            nc.sync.dma_start(idxr[:, k, :], rep_ap)

    gpool = ctx.enter_context(tc.tile_pool(name="gather", bufs=3))
    dpool = ctx.enter_context(tc.tile_pool(name="ps_deform", bufs=1, space="PSUM"))
    wtpool = ctx.enter_context(tc.tile_pool(name="ps_wt", bufs=1, space="PSUM"))
    gsems = [nc.alloc_semaphore(f"gdma{i}") for i in range(8)]
    csem = nc.alloc_semaphore("gcons")
    gtiles = {}

    def gwait(i):
        return gsems[i % 8], 16 * (i // 8 + 1)

    def emit_prep(i):
        """Prep gather i's descriptors on Q7 core pair i%4 (parallel gen)."""
        q, k = divmod(i, 9)
        _, idxr = qtiles[q]
        G = gpool.tile([128, 4, QP], F16)
        gtiles[i] = G
        nc.gpsimd.dma_gather(
            out_ap=G[:],
            in_ap=io["src4"],
            idxs_ap=idxr[:, k, :],
            num_idxs=QP,
            num_idxs_reg=QP,
            elem_size=512,
            transpose=True,
            single_packet=False,
            queue_num=i % 4,
            prepare_only=True,
            sem=gsems[i % 8],
        )

    # The actual gather DMAs are strictly serialized — concurrent
    # transpose gathers corrupt the shared xbar spray.  trigger(i) waits
    # for dma-complete(i-1) and for the consumers of the G buffer it
    # overwrites (csem, gpool bufs=3).  Preps run 3 gathers ahead so the
    # four Q7 pairs generate descriptors concurrently.  Tile treats
    # explicit-sem preps as user-synced, so consumer waits are manual.
    dpss = {}

    def stage_DE(q):
        frac3, idxr = qtiles[q]
        dps = dpool.tile([128, QP], F32)  # 4 PSUM banks
        for k in range(9):
            i = 9 * q + k
            if i == 0:
                emit_prep(0)
                emit_prep(1)
                emit_prep(2)
            if i >= 3:
                nc.gpsimd.wait_ge(csem, i - 2)
            if i >= 1:
                ps, pv = gwait(i - 1)
                nc.gpsimd.wait_ge(ps, pv)
            nc.gpsimd.trigger_dma(count=None, queue_num=i % 4)
            if i + 3 < 36:
                emit_prep(i + 3)
            G = gtiles.pop(i)
            s, v = gwait(i)
            nc.vector.wait_ge(s, v)
            nc.tensor.wait_ge(s, v)
            for cc in range(4):
                stg = stgp.tile([1, 3, 512], F16, tag="stg")
                nc.sync.dma_start(stg[:], frac3[k : k + 1, :, ts(cc, 512)])
                wt = wtpool.tile([128, 3, 512], F32, tag="wt")
                for m in range(3):
                    rhs = _ap(stg[:], m * 512, [list(stg[:].ap[0]), [1, 512]])
                    nc.tensor.matmul(
                        wt[:, m, :], lhsT=ones_sb[:], rhs=rhs, start=True, stop=True
                    )
                # weight the three difference planes in place (one DVE op)
                nc.vector.tensor_tensor(
                    G[:, 1:4, ts(cc, 512)], G[:, 1:4, ts(cc, 512)], wt[:],
                    Alu.mult,
                )
                for m in range(4):
                    nc.tensor.matmul(
                        dps[:, ts(cc, 512)],
                        lhsT=w_def_sb[:, k, :],
                        rhs=G[:, m, ts(cc, 512)],
                        start=(k == 0 and m == 0),
                        stop=(k == 8 and m == 3),
                    )
            nc.tensor.sem_inc(csem, 1)  # consumers of G(i) done
        qtiles.pop(q)
        # ---------------- stage E: aligned*gate + high ----------------
        for cc in range(4):
            gsl = ts(q * 4 + cc, 512)
            t1 = spool.tile([128, 512], F32, tag="as1")
            nc.vector.scalar_tensor_tensor(
                out=t1[:], in0=dps[:, ts(cc, 512)], scalar=b_def_sb[:],
                in1=gate_sb[:, gsl], op0=Alu.add, op1=Alu.mult,
            )
            hc = s1pool.tile([128, 512], F32, tag="hc")
            nc.sync.dma_start(hc[:], io["highc"][:, gsl])
            nc.vector.tensor_tensor(t1[:], t1[:], hc[:], Alu.add)
            nc.sync.dma_start(out_d[:, gsl], t1[:])

    # software pipeline: A/B one quarter ahead of D/E
    stage_AB(0)
    stage_AB(1)
    stage_DE(0)
    stage_AB(2)
    stage_DE(1)
    stage_AB(3)
    stage_DE(2)
    stage_DE(3)

    ctx.close()


# ======================= host side =======================

def _prep_shared(w_off, b_off, w_def, b_def, w_mod, b_mod):
    w_off_g = w_off[PERM]                      # [18, 256, 3, 3]
    w_off_t = np.ascontiguousarray(
        w_off_g.reshape(18, 2, 128, 3, 3).transpose(1, 3, 4, 2, 0)
    ).astype(np.float16)                       # [2,3,3,128,18]
    b_off_g = b_off[PERM].reshape(18, 1).astype(np.float32)
    w_def_t = np.ascontiguousarray(
        w_def.reshape(128, 128, 9).transpose(2, 1, 0)
    ).astype(np.float16)                       # [9, c, o]
    w_mod_t = np.ascontiguousarray(
        w_mod.reshape(128, 2, 128).transpose(1, 2, 0)
    ).astype(np.float16)                       # [2, c, o]
    return dict(
        w_off_t=w_off_t,
        b_off_g=b_off_g,
        w_def_t=w_def_t,
        b_def_c=b_def.reshape(128, 1).astype(np.float32),
        w_mod_t=w_mod_t,
        b_mod_c=b_mod.reshape(128, 1).astype(np.float32),
    )


def _prep_src4(low_b):
    """4-plane pixel-major monomial table of the guard-padded low image."""
    xp = np.zeros((C, HP, HP), np.float32)
    xp[:, GP : GP + H, GP : GP + W] = low_b
    f = xp.reshape(C, NP)
    p0 = f
    p1 = np.zeros_like(f)
    p1[:, :-1] = f[:, 1:] - f[:, :-1]
    p2 = np.zeros_like(f)
    p2[:, :-HP] = f[:, HP:] - f[:, :-HP]
    p3 = np.zeros_like(f)
    p3[:, : -HP - 1] = f[:, HP + 1 :] - f[:, HP:-1] - f[:, 1 : -HP] + f[:, : -HP - 1]
    planes = np.stack([p0, p1, p2, p3], 0)      # [4, C, NP]
    return np.ascontiguousarray(planes.transpose(2, 0, 1)).astype(
        np.float16
    ).reshape(NP, 512)


def _prep_base(h0):
    """Sampling-position bases, wrapped cols; [18, NQ, QP]
    (rows 0-8 = py bases per tap, rows 9-17 = px bases)."""
    base = np.empty((18, NQ, QP), np.float32)
    u = np.arange(QP)
    p16 = u // 128
    s = u % 128
    jloc = s * 16 + p16
    for q in range(NQ):
        j = q * QP + jloc
        h = h0 + j // W
        w = j % W
        for k in range(9):
            ky, kx = k // 3, k % 3
            base[k, q] = h + (ky - 1) + GP
            base[9 + k, q] = w + (kx - 1) + GP
    return base.astype(np.float16)


def _prep_core(low_b, high_b, h0):
    lp = np.pad(low_b, ((0, 0), (1, 1), (1, 1)))
    hp = np.pad(high_b, ((0, 0), (1, 1), (1, 1)))
    lowp = np.ascontiguousarray(lp[:, h0 : h0 + 66, :]).reshape(128, -1).astype(
        np.float16
    )
    highp = np.ascontiguousarray(hp[:, h0 : h0 + 66, :]).reshape(128, -1).astype(
        np.float16
    )
    highc = np.ascontiguousarray(high_b[:, h0 : h0 + HR, :]).reshape(128, -1).astype(
        np.float32
    )
    return lowp, highp, highc


_PROGRAM_CACHE = {}
_LAST_IN_MAPS = None


def make_in_maps(low_res, high_res, w_off, b_off, w_def, b_def, w_mod, b_mod):
    shared = _prep_shared(
        np.asarray(w_off, np.float32), np.asarray(b_off, np.float32),
        np.asarray(w_def, np.float32), np.asarray(b_def, np.float32),
        np.asarray(w_mod, np.float32), np.asarray(b_mod, np.float32),
    )
    low_res = np.asarray(low_res, np.float32)
    high_res = np.asarray(high_res, np.float32)
    src4_by_batch = [_prep_src4(low_res[b]) for b in range(B)]
    in_maps = []
    for core in range(8):
        b, half = core // 2, core % 2
        h0 = half * HR
        lowp, highp, highc = _prep_core(low_res[b], high_res[b], h0)
        m = dict(shared)
        m["src4"] = src4_by_batch[b]
        m["lowp"] = lowp
        m["highp"] = highp
        m["highc"] = highc
        m["base_w2"] = _prep_base(h0)
        in_maps.append(m)
    return in_maps


def kernel(low_res, high_res, w_off, b_off, w_def, b_def, w_mod, b_mod):
    global _LAST_IN_MAPS
    if "nc" not in _PROGRAM_CACHE:
        _PROGRAM_CACHE["nc"] = build_program()
    nc = _PROGRAM_CACHE["nc"]

    in_maps = make_in_maps(
        low_res, high_res, w_off, b_off, w_def, b_def, w_mod, b_mod
    )
    _LAST_IN_MAPS = in_maps

    from concourse import bass_utils

    res = bass_utils.run_bass_kernel_spmd(nc, in_maps, core_ids=list(range(8)))
    out = np.empty((B, C, H, W), np.float32)
    for core in range(8):
        b, half = core // 2, core % 2
        out[b, :, half * HR : half * HR + HR, :] = (
            res.results[core]["out"].reshape(C, HR, W)
        )
    return out
